# revision 18
# baseline (speedup 1.0000x reference)
"""Trainium2 Bass kernel for nn_CConv (causal depthwise FFT-conv, 512 taps).

The reference's FFT conv is exactly a causal depthwise conv1d with a
512-step learned init state prepended:
    out[b,t,c] = sum_k f[k,c] * xc[b, t+512-k, c],  xc = concat(init, x)

Mapping to the tensor engine (per channel c):
    out[i, (m0,b)] = sum_q  W_q[j,i].T @ X[j, (m0+q, b)]
where W_q[j,i] = f[i-j+128*(4-q), c] are 5 Toeplitz slices of a
[128 x 640] band, and X is the natural time-tiled x (partition = t%128,
columns = (time chunk, batch)).  The 5 matmuls accumulate in PSUM.
Channels are sharded 8 ways across cores; inside a core, channels
stream in groups of 8 (fused x+band slab DMA -> 40 matmuls -> PSUM->
SBUF cast -> batched output DMA).

HBM traffic is the bottleneck, so: operands are fp16 (filter prescaled
by 32, x by 1/32 — exact powers of two keeping both far from fp16
subnormals; fp32 PSUM accumulation), the output is written fp16 and
upcast on host, and one channel per group has its Toeplitz band
generated on-chip by an SBUF->SBUF SWDGE DMA with a negative-stride
(diagonal) source AP over the padded filter — trading spare fabric
bandwidth for HBM bytes.
"""

import os

import numpy as np

import concourse.bacc as bacc
import concourse.mybir as mybir
from concourse.bass import AP
from concourse.bass_utils import run_bass_kernel_spmd
from concourse.tile import TileContext

B, L, D, CLEN = 4, 4096, 1024, 512
NCORES = 8
DSH = D // NCORES            # 128 channels per core
GCH = 8                      # channels per group
NG = DSH // GCH              # 16 groups per core
MIN = (CLEN + L) // 128      # 36 input chunks (4 init + 32 x)
MOUT = L // 128              # 32 output blocks
NQ = 5                       # contraction chunks per output block
XW = MIN * B                 # 144 x columns per channel
BW = 640                     # band columns per channel
NSHIP = GCH - 1              # channels per group whose band ships from HBM
SLABW = GCH * XW + NSHIP * BW    # 5632 shipped columns per slab row
CW = SLABW + BW                  # 6272 slab columns incl generated band
PFW = 767                    # padded filter length
SCALE = 32.0

_CACHE = {}
LAST_RESULTS = None          # BassKernelResults of the most recent run


def _xoff(c):
    return c * XW


def _boff(c):
    return GCH * XW + c * BW if c < NSHIP else SLABW


def _build_bass():
    # Bacc (not plain Bass): its compile() legalizes sync waits (>1 wait per
    # instruction gets split into InstEventSemaphore), which walrus requires.
    nc = bacc.Bacc(None, target_bir_lowering=False)
    f16, f32 = mybir.dt.float16, mybir.dt.float32
    sd = nc.declare_dram_parameter("slab", [NG, 128, SLABW], f16, isOutput=False)
    pfd = nc.declare_dram_parameter("pf", [NG, PFW], f16, isOutput=False)
    od = nc.declare_dram_parameter("out", [NG, 128, GCH, 128], f16, isOutput=True)

    with TileContext(nc) as tc:
        with (
            tc.tile_pool(name="pfp", bufs=1) as pfp,
            tc.tile_pool(name="sp", bufs=4) as sp,
            tc.tile_pool(name="op", bufs=4) as op,
            tc.tile_pool(name="pp", bufs=4, space="PSUM") as pp,
        ):
            pft = pfp.tile([NG, PFW], f16)
            nc.sync.dma_start(out=pft[:], in_=pfd[:])
            batches = [(0, 4), (4, 4), (8, 4), (12, 2), (14, 1), (15, 1)]
            for g0, nb in batches:
                ot = op.tile([128, nb, GCH, 128], f16)
                for k in range(nb):
                    g = g0 + k
                    st = sp.tile([128, CW], f16)
                    if g == 0:
                        # split so the first channels' matmuls start sooner
                        nc.sync.dma_start(
                            out=st[:, : 2 * XW], in_=sd[g, :, : 2 * XW]
                        )
                        nc.sync.dma_start(
                            out=st[:, 2 * XW : SLABW], in_=sd[g, :, 2 * XW :]
                        )
                    else:
                        nc.sync.dma_start(out=st[:, :SLABW], in_=sd[g])
                    # on-chip Toeplitz expansion for the last channel:
                    # band[j, u] = pf[g, 127 - j + u] via a diagonal source AP
                    pfa = pft[:]
                    diag = AP(
                        pfa.tensor,
                        pfa.offset + g * PFW + 127,
                        [[PFW, 1], [-1, 128], [1, BW]],
                    )
                    nc.gpsimd.dma_start(out=st[:, SLABW:], in_=diag)
                    for c in range(GCH):
                        ps = pp.tile([128, 128], f32)
                        bo = _boff(c)
                        xo = _xoff(c)
                        for q in range(NQ):
                            nc.tensor.matmul(
                                ps[:],
                                lhsT=st[:, bo + 128 * (4 - q) : bo + 128 * (5 - q)],
                                rhs=st[:, xo + 4 * q : xo + 4 * q + 128],
                                start=(q == 0),
                                stop=(q == NQ - 1),
                            )
                        nc.vector.tensor_copy(out=ot[:, k, c, :], in_=ps[:])
                nc.sync.dma_start(
                    out=od[g0 : g0 + nb].transpose([1, 0, 2, 3]), in_=ot[:]
                )
    nc.finalize()  # Bacc.compile(): reg alloc + sync-wait legalization
    return nc


def _prep_inputs(x, last_input_init, filt):
    """Host-side: cast/scale to fp16 and prearrange into the exact SBUF
    layout so every DMA is a contiguous line-rate copy.

    slab[core, g, j, c*144 + m*4 + b]            = xc[b, 128*m + j, ch] / 32
    slab[core, g, j, 1152 + c*640 + u] (c < 7)   = 32*f[u - j, ch]
    pf[core, g, 127 + k]                         = 32*f[k, ch7(g)]
    """
    x = np.asarray(x, dtype=np.float32)
    init = np.asarray(last_input_init, dtype=np.float32)
    filt = np.asarray(filt, dtype=np.float32)

    xc = np.concatenate(
        [np.broadcast_to(init[None], (B, CLEN, D)), x], axis=1
    )  # [B, 4608, D]
    xh = (xc * np.float32(1.0 / SCALE)).astype(np.float16)
    xr = xh.reshape(B, MIN, 128, D)                      # [b, m, j, ch]
    xt = xr.transpose(3, 2, 1, 0)                        # [ch, j, m, b]
    xt = np.ascontiguousarray(xt).reshape(D, 128, XW)    # [ch, j, m*4+b]

    fs = (filt * np.float32(SCALE)).astype(np.float16)   # [512, D]
    pf = np.zeros((D, PFW), np.float16)
    pf[:, 127:639] = fs.T
    jj = np.arange(128)
    uu = np.arange(BW)
    idx = 127 - jj[:, None] + uu[None, :]                # [128, 640] in [0, 767)
    band = pf[:, idx]                                    # [ch, j, u]

    xg = xt.reshape(NCORES, NG, GCH, 128, XW)
    bg = band.reshape(NCORES, NG, GCH, 128, BW)
    slab = np.empty((NCORES, NG, 128, SLABW), np.float16)
    slab[:, :, :, : GCH * XW] = (
        xg.transpose(0, 1, 3, 2, 4).reshape(NCORES, NG, 128, GCH * XW)
    )
    slab[:, :, :, GCH * XW :] = (
        bg[:, :, :NSHIP].transpose(0, 1, 3, 2, 4).reshape(NCORES, NG, 128, NSHIP * BW)
    )
    pfg = pf.reshape(NCORES, NG, GCH, PFW)[:, :, GCH - 1]  # [core, g, 767]
    return slab, np.ascontiguousarray(pfg)


def kernel(x, last_input_init, filt):
    global LAST_RESULTS
    if "nc" not in _CACHE:
        _CACHE["nc"] = _build_bass()
    nc = _CACHE["nc"]

    slab, pfg = _prep_inputs(x, last_input_init, filt)
    in_maps = [{"slab": slab[core], "pf": pfg[core]} for core in range(NCORES)]

    trace = bool(os.environ.get("BASS_TRACE"))
    res = run_bass_kernel_spmd(nc, in_maps, list(range(NCORES)), trace=trace)
    LAST_RESULTS = res

    outs = []
    for core in range(NCORES):
        o = res.results[core]["out"].astype(np.float32)  # [NG, 128, GCH, 128]
        o = o.reshape(NG, 128, GCH, MOUT, B)             # [g, i, c, m0, b]
        o = o.transpose(4, 3, 1, 0, 2)                   # [b, m0, i, g, c]
        outs.append(o.reshape(B, L, DSH))
    out = np.concatenate(outs, axis=2)
    return np.ascontiguousarray(out, dtype=np.float32)


# revision 23
# speedup vs baseline: 1.7822x; 1.7822x over previous
"""Trainium2 Bass kernel for nn_CConv (causal depthwise FFT-conv, 512 taps).

The reference's FFT conv is exactly a causal depthwise conv1d with a
512-step learned init state prepended:
    out[b,t,c] = sum_k f[k,c] * xc[b, t+512-k, c],  xc = concat(init, x)

Mapping to the tensor engine (per channel c):
    out[i, (m0,b)] = sum_q  W_q[j,i].T @ X[j, (m0+q, b)]
where W_q[j,i] = f[i-j+128*(4-q), c] are 5 Toeplitz slices of a
[128 x 640] band, and X is the natural time-tiled x (partition = t%128,
columns = (time chunk, batch)).  The 5 matmuls accumulate in PSUM.
Channels are sharded 8 ways across cores; inside a core, channels
stream in groups of 8 (fused x+band slab DMA -> 40 matmuls -> PSUM->
SBUF cast -> batched output DMA).

HBM traffic is the bottleneck, so: operands are fp16 (filter prescaled
by 32, x by 1/32 — exact powers of two keeping both far from fp16
subnormals; fp32 PSUM accumulation), the output is written fp16 and
upcast on host, and one channel per group has its Toeplitz band
generated on-chip by an SBUF->SBUF SWDGE DMA with a negative-stride
(diagonal) source AP over the padded filter — trading spare fabric
bandwidth for HBM bytes.
"""

import os

import numpy as np

import concourse.bacc as bacc
import concourse.mybir as mybir
from concourse.bass import AP
from concourse.bass_utils import run_bass_kernel_spmd
from concourse.tile import TileContext

B, L, D, CLEN = 4, 4096, 1024, 512
NCORES = 8
DSH = D // NCORES            # 128 channels per core
GCH = 8                      # channels per group
NG = DSH // GCH              # 16 groups per core
MIN = (CLEN + L) // 128      # 36 input chunks (4 init + 32 x)
MOUT = L // 128              # 32 output blocks
NQ = 5                       # contraction chunks per output block
XW = MIN * B                 # 144 x columns per channel
BW = 640                     # band columns per channel
NSHIP = GCH                  # channels per group whose band ships from HBM
SLABW = GCH * XW + NSHIP * BW    # 5632 shipped columns per slab row
CW = SLABW + BW                  # 6272 slab columns incl generated band
PFW = 767                    # padded filter length
SCALE = 32.0

_CACHE = {}
LAST_RESULTS = None          # BassKernelResults of the most recent run


def _xoff(c):
    return c * XW


def _boff(c):
    return GCH * XW + c * BW if c < NSHIP else SLABW


def _build_bass():
    # Bacc (not plain Bass): its compile() legalizes sync waits (>1 wait per
    # instruction gets split into InstEventSemaphore), which walrus requires.
    nc = bacc.Bacc(None, target_bir_lowering=False)
    f16, f32 = mybir.dt.float16, mybir.dt.float32
    sd = nc.declare_dram_parameter("slab", [NG, 128, SLABW], f16, isOutput=False)
    if NSHIP < GCH:
        pfd = nc.declare_dram_parameter("pf", [128, PFW], f16, isOutput=False)
    od = nc.declare_dram_parameter("out", [NG, 128, GCH, 128], f16, isOutput=True)

    with TileContext(nc) as tc:
        with (
            tc.tile_pool(name="pfp", bufs=1) as pfp,
            tc.tile_pool(name="sp", bufs=4) as sp,
            tc.tile_pool(name="op", bufs=4) as op,
            tc.tile_pool(name="pp", bufs=4, space="PSUM") as pp,
        ):
            if NSHIP < GCH:
                pft = pfp.tile([128, PFW], f16)
                nc.sync.dma_start(out=pft[:], in_=pfd[:])
            batches = [(0, 4), (4, 4), (8, 4), (12, 2), (14, 1), (15, 1)]
            for g0, nb in batches:
                ot = op.tile([128, nb, GCH, 128], f16)
                for k in range(nb):
                    g = g0 + k
                    st = sp.tile([128, CW], f16)
                    if g == 0:
                        # split so the first channels' matmuls start sooner
                        nc.sync.dma_start(
                            out=st[:, : 2 * XW], in_=sd[g, :, : 2 * XW]
                        )
                        nc.sync.dma_start(
                            out=st[:, 2 * XW : SLABW], in_=sd[g, :, 2 * XW :]
                        )
                    else:
                        nc.sync.dma_start(out=st[:, :SLABW], in_=sd[g])
                    if NSHIP < GCH:
                        # on-chip Toeplitz expansion for the last channel via
                        # two SWDGE DMAs with diagonal (negative-stride) source
                        # APs; the two pre-shifted pf replicas live on ports
                        # of opposite parity so the reads spread.
                        sta = st[:]
                        pfa = pft[:]
                        for rho in range(2):
                            src = AP(
                                pfa.tensor,
                                pfa.offset + (rho * 64 + 4 * (g % 16)) * PFW + 126,
                                [[PFW, 1], [-2, 64], [1, BW]],
                            )
                            dst = AP(
                                sta.tensor,
                                sta.offset + rho * CW + SLABW,
                                [[2 * CW, 64], [1, BW]],
                            )
                            nc.gpsimd.dma_start(out=dst, in_=src)
                    for c in range(GCH):
                        ps = pp.tile([128, 128], f32)
                        bo = _boff(c)
                        xo = _xoff(c)
                        for q in range(NQ):
                            nc.tensor.matmul(
                                ps[:],
                                lhsT=st[:, bo + 128 * (4 - q) : bo + 128 * (5 - q)],
                                rhs=st[:, xo + 4 * q : xo + 4 * q + 128],
                                start=(q == 0),
                                stop=(q == NQ - 1),
                            )
                        nc.vector.tensor_copy(out=ot[:, k, c, :], in_=ps[:])
                nc.sync.dma_start(
                    out=od[g0 : g0 + nb].transpose([1, 0, 2, 3]), in_=ot[:]
                )
    nc.finalize()  # Bacc.compile(): reg alloc + sync-wait legalization
    return nc


def _prep_inputs(x, last_input_init, filt):
    """Host-side: cast/scale to fp16 and prearrange into the exact SBUF
    layout so every DMA is a contiguous line-rate copy.

    slab[core, g, j, c*144 + m*4 + b]            = xc[b, 128*m + j, ch] / 32
    slab[core, g, j, 1152 + c*640 + u] (c < 7)   = 32*f[u - j, ch]
    pf[core, g, 127 + k]                         = 32*f[k, ch7(g)]
    """
    x = np.asarray(x, dtype=np.float32)
    init = np.asarray(last_input_init, dtype=np.float32)
    filt = np.asarray(filt, dtype=np.float32)

    xc = np.concatenate(
        [np.broadcast_to(init[None], (B, CLEN, D)), x], axis=1
    )  # [B, 4608, D]
    xh = (xc * np.float32(1.0 / SCALE)).astype(np.float16)
    xr = xh.reshape(B, MIN, 128, D)                      # [b, m, j, ch]
    xt = xr.transpose(3, 2, 1, 0)                        # [ch, j, m, b]
    xt = np.ascontiguousarray(xt).reshape(D, 128, XW)    # [ch, j, m*4+b]

    fs = (filt * np.float32(SCALE)).astype(np.float16)   # [512, D]
    pf = np.zeros((D, PFW), np.float16)
    pf[:, 127:639] = fs.T
    jj = np.arange(128)
    uu = np.arange(BW)
    idx = 127 - jj[:, None] + uu[None, :]                # [128, 640] in [0, 767)
    band = pf[:, idx]                                    # [ch, j, u]

    xg = xt.reshape(NCORES, NG, GCH, 128, XW)
    bg = band.reshape(NCORES, NG, GCH, 128, BW)
    slab = np.empty((NCORES, NG, 128, SLABW), np.float16)
    slab[:, :, :, : GCH * XW] = (
        xg.transpose(0, 1, 3, 2, 4).reshape(NCORES, NG, 128, GCH * XW)
    )
    slab[:, :, :, GCH * XW :] = (
        bg[:, :, :NSHIP].transpose(0, 1, 3, 2, 4).reshape(NCORES, NG, 128, NSHIP * BW)
    )
    if NSHIP == GCH:
        return slab, None
    # pre-shifted pf replicas for the on-chip band generation: the replica
    # rho of group g's last channel sits at partition rho*64 + 4g, shifted
    # so the generation DMA's read offset is rho-independent.
    pfc = pf.reshape(NCORES, NG, GCH, PFW)[:, :, GCH - 1]  # [core, g, 767]
    pf2 = np.zeros((NCORES, 128, PFW), np.float16)
    for rho in range(2):
        shifted = np.zeros((NCORES, NG, PFW), np.float16)
        if rho == 0:
            shifted[:, :, : PFW - 1] = pfc[:, :, 1:]
        else:
            shifted[:, :, :] = pfc
        pf2[:, rho * 64 + 4 * np.arange(NG)] = shifted
    return slab, pf2


def kernel(x, last_input_init, filt):
    global LAST_RESULTS
    if "nc" not in _CACHE:
        _CACHE["nc"] = _build_bass()
    nc = _CACHE["nc"]

    slab, pfg = _prep_inputs(x, last_input_init, filt)
    in_maps = [
        {"slab": slab[core]}
        | ({} if pfg is None else {"pf": pfg[core]})
        for core in range(NCORES)
    ]

    trace = bool(os.environ.get("BASS_TRACE"))
    res = run_bass_kernel_spmd(nc, in_maps, list(range(NCORES)), trace=trace)
    LAST_RESULTS = res

    outs = []
    for core in range(NCORES):
        o = res.results[core]["out"].astype(np.float32)  # [NG, 128, GCH, 128]
        o = o.reshape(NG, 128, GCH, MOUT, B)             # [g, i, c, m0, b]
        o = o.transpose(4, 3, 1, 0, 2)                   # [b, m0, i, g, c]
        outs.append(o.reshape(B, L, DSH))
    out = np.concatenate(outs, axis=2)
    return np.ascontiguousarray(out, dtype=np.float32)
